# revision 2
# baseline (speedup 1.0000x reference)
"""TRN2 Bass kernel v2 for nn_CropLayer (crop_and_resize, bilinear, 28x28).

Contract: kernel(images, boxes) takes the FULL inputs
  images [8, 512, 512, 32] f32, boxes [8, 100, 4] f32
and returns the FULL output [800, 28, 28, 32] f32, running on 8 NeuronCores
(data-parallel over batch, one image per core).

v2 design: the host repacks each image into two row-pair-interleaved copies
  P0[yp, x, r, c] = img[2*yp + r, x, c]          (even y0 pairs)
  P1[yp, x, r, c] = img[min(2*yp + 1 + r, 511), x, c]  (odd y0 pairs)
so the 2x2xC bilinear neighbourhood of any sample point is two 256 B units
at CONSECUTIVE unit addresses (unit = one x position = 2 rows x 32 ch).
Each point costs two 256 B gather descriptors (x0 unit and x1 unit) - half
the bytes of v1's two 512 B windows - and the blend needs only 4 weights:
  out = w_tl*X0.s0 + w_bl*X0.s1 + w_tr*X1.s0 + w_br*X1.s1   (7 DVE ops).

int16 gather indices cover one band of 64 yp values (64*512 units = 2^15);
points are host-sorted by (copy, band) into 8 classes, padded to /128, and
chunked; the host unsorts the device output.
"""

import sys

if '/opt/trn_rl_repo' not in sys.path:
    sys.path.insert(0, '/opt/trn_rl_repo')

import numpy as np

import concourse.bacc as bacc
import concourse.mybir as mybir
import concourse.tile as tile
import concourse.tile_sem_assignment as tsa
from concourse.ap import AP

F32 = mybir.dt.float32
I16 = mybir.dt.int16
OP = mybir.AluOpType

P = 128
H = 512
W = 512
C = 32
CROP = 28
NB = 100
PTS = CROP * CROP
NPT = NB * PTS
CHUNK = 6272              # points per chunk (= 128 * 49)
MCOL = CHUNK // P
NCLS = 8                  # (copy in {0,1}) x (band in {0..3})
YP_BAND = 64              # yp values per band; 64*512 = 32768 unit addrs
UNIT = 2 * C              # one x position: 2 rows x C channels (256 B)
COPY_ELEMS = (H // 2) * W * 2 * C   # 8.39 M f32 per repacked copy

NQ = 4                    # SWDGE queues
SINGLE_PACKET = False     # dma_gather packetization mode
MERGED = True             # one 512B desc per point (vs two 256B descs)
SORT_BY_ADDR = True       # order points by image address within a class

# ---------------------------------------------------------------------------
# Tile round-robins Pool-engine DMA insts over all 8 DMASW sem lanes with no
# regard for the SWDGE queue they run on, but each lane may only be updated
# from one queue (ucode constraint, enforced by CoreSim).  Patch the lane
# assignment so each queue gets a disjoint lane set.
_orig_assign_tick = tsa.TileClockTick._assign_tick
_IDXQ = {}


def _queue_aware_assign_tick(self, inst):
    qn = getattr(inst, "queue_num", None)
    if isinstance(inst, mybir.InstDMAGatherAnt) and qn is not None:
        ctr = _IDXQ.setdefault(id(self), {})
        c = ctr.get(qn, 0)
        ctr[qn] = c + 1
        step = 4 if NQ > 2 else 2
        # keep queues on disjoint lane sets for ANY lane count (loop
        # stages use 5 lanes instead of 8)
        n = self.swdge_sem_count
        slots = max(1, n // step)
        self.next_sw_dma_idx = (qn + step * (c % slots)) % n
    return _orig_assign_tick(self, inst)


if tsa.TileClockTick._assign_tick.__name__ != "_queue_aware_assign_tick":
    tsa.TileClockTick._assign_tick = _queue_aware_assign_tick


# ---------------------------------------------------------------------------
def _host_point_data(boxes_core):
    """Per-core box math in f32, mirroring the reference op-for-op.

    Returns, per (box, crop_y) unit of CROP points:
      cls   [NB, CROP]        class id = copy*4 + band
      idx0  [NB, CROP, CROP]  band-local unit index of x0 (int16)
      idx1  [NB, CROP, CROP]  band-local unit index of x1
      w4    [NB, CROP, CROP, 4]  folded blend weights (tl, bl, tr, br)
    """
    b = boxes_core.astype(np.float32)
    y1, x1, y2, x2 = b[:, 0], b[:, 1], b[:, 2], b[:, 3]
    g = np.arange(CROP, dtype=np.float32)
    hsc = (y2 - y1) * np.float32(H - 1) / np.float32(CROP - 1)
    wsc = (x2 - x1) * np.float32(W - 1) / np.float32(CROP - 1)
    in_y = y1[:, None] * np.float32(H - 1) + g[None, :] * hsc[:, None]
    in_x = x1[:, None] * np.float32(W - 1) + g[None, :] * wsc[:, None]

    vy = (in_y >= 0) & (in_y <= H - 1)
    vx = (in_x >= 0) & (in_x <= W - 1)
    y0f = np.floor(in_y)
    x0f = np.floor(in_x)
    ly = in_y - y0f
    lx = in_x - x0f
    y0 = np.clip(y0f.astype(np.int32), 0, H - 1)
    x0 = np.clip(x0f.astype(np.int32), 0, W - 1)
    x1i = np.minimum(x0 + 1, W - 1)

    copy = (y0 & 1).astype(np.int32)           # even y0 -> P0, odd -> P1
    yp = (y0 - copy) >> 1                      # row-pair index in its copy
    band = yp // YP_BAND
    ylocal = yp - band * YP_BAND
    cls = copy * 4 + band                      # [NB, CROP]

    base = (ylocal * W).astype(np.int32)       # [NB, CROP]
    idx0 = (base[:, :, None] + x0[:, None, :]).astype(np.int16)
    idx1 = (base[:, :, None] + x1i[:, None, :]).astype(np.int16)

    mask = (vy[:, :, None] & vx[:, None, :]).astype(np.float32)
    omly = (1 - ly)[:, :, None]
    omlx = (1 - lx)[:, None, :]
    w_tl = omly * omlx
    w_bl = ly[:, :, None] * omlx
    w_tr = omly * lx[:, None, :]
    w_br = ly[:, :, None] * lx[:, None, :]
    w4 = np.stack([w_tl, w_bl, w_tr, w_br], axis=-1) * mask[:, :, :, None]
    return cls, idx0, idx1, w4.astype(np.float32)


def _repack(img):
    """img [H, W, C] f32 -> (P0, P1) flat row-pair-interleaved copies."""
    p0 = np.ascontiguousarray(
        img.reshape(H // 2, 2, W, C).transpose(0, 2, 1, 3)).ravel()
    shifted = np.concatenate([img[1:], img[H - 1:]], axis=0)
    p1 = np.ascontiguousarray(
        shifted.reshape(H // 2, 2, W, C).transpose(0, 2, 1, 3)).ravel()
    return p0, p1


def _make_schedule(images, boxes):
    B = images.shape[0]
    per_core = []
    cls_counts = np.zeros((B, NCLS), np.int64)
    for c in range(B):
        cls, i0, i1, w4 = _host_point_data(boxes[c])
        per_core.append((cls, i0, i1, w4))
        for k in range(NCLS):
            cls_counts[c, k] = int((cls == k).sum()) * CROP

    pk = cls_counts.max(axis=0)
    pk = ((pk + P - 1) // P) * P
    total = int(pk.sum())
    L = ((total + CHUNK - 1) // CHUNK) * CHUNK
    last = int(np.nonzero(pk)[0][-1]) if pk.sum() else 0
    pk[last] += L - total

    segments = [[] for _ in range(L // CHUNK)]
    off = 0
    for k in range(NCLS):
        remaining = int(pk[k])
        while remaining > 0:
            ch = off // CHUNK
            room = CHUNK - (off % CHUNK)
            take = min(room, remaining)
            segments[ch].append((k, off % CHUNK, take))
            off += take
            remaining -= take

    in_maps = []
    unsort_rows = []
    for c in range(B):
        cls, i0, i1, w4 = per_core[c]
        # per-point flat views; point id = ((n*CROP + iy)*CROP + ix)
        cls_pt = np.repeat(cls.ravel(), CROP)          # [NPT]
        i0_pt = i0.reshape(-1)
        i1_pt = i1.reshape(-1)
        w4_pt = w4.reshape(-1, 4)
        flat0 = np.zeros(L, np.int16)
        flat1 = np.zeros(L, np.int16)
        flatW = np.zeros((L, 4), np.float32)
        pos_of = np.empty(NPT, np.int64)
        off = 0
        for k in range(NCLS):
            pts = np.nonzero(cls_pt == k)[0]
            if SORT_BY_ADDR and len(pts):
                pts = pts[np.argsort(i0_pt[pts], kind='stable')]
            npts = len(pts)
            if npts:
                sl = slice(off, off + npts)
                flat0[sl] = i0_pt[pts]
                flat1[sl] = i1_pt[pts]
                flatW[sl] = w4_pt[pts]
                pos_of[pts] = np.arange(off, off + npts)
            off += int(pk[k])
        wrap0 = flat0.reshape(L // 16, 16).T
        wrap1 = flat1.reshape(L // 16, 16).T
        idx0 = np.tile(wrap0, (8, 1)).copy()
        idx1 = np.tile(wrap1, (8, 1)).copy()
        wts = np.ascontiguousarray(
            flatW.reshape(L // P, P, 4).transpose(1, 0, 2).reshape(P, -1))
        p0, p1 = _repack(images[c])
        pad = np.zeros(UNIT, np.float32)   # MERGED reads 1 unit past the end
        p0 = np.concatenate([p0, pad])
        p1 = np.concatenate([p1, pad])
        in_maps.append({"p0": p0, "p1": p1, "idx0": idx0, "idx1": idx1,
                        "wts": wts})

        q = pos_of
        ch = q // CHUNK
        ql = q % CHUNK
        unsort_rows.append(ch * CHUNK + (ql % P) * MCOL + ql // P)

    return in_maps, segments, unsort_rows, L


def _build_nc(segments, L, num_devices=8, repeat=1):
    nc = bacc.Bacc("TRN2", target_bir_lowering=False, debug=False,
                   num_devices=num_devices, num_swdge_queues=NQ)
    p0_d = nc.dram_tensor("p0", [COPY_ELEMS + UNIT], F32,
                          kind="ExternalInput")
    p1_d = nc.dram_tensor("p1", [COPY_ELEMS + UNIT], F32,
                          kind="ExternalInput")
    idx0_d = nc.dram_tensor("idx0", [P, L // 16], I16, kind="ExternalInput")
    idx1_d = nc.dram_tensor("idx1", [P, L // 16], I16, kind="ExternalInput")
    wts_d = nc.dram_tensor("wts", [P, (L // P) * 4], F32, kind="ExternalInput")
    out_d = nc.dram_tensor("out", [L, C], F32, kind="ExternalOutput")

    nchunks = L // CHUNK
    outv = out_d.ap().rearrange("(c p q) e -> c p (q e)", c=nchunks, p=P)

    def class_in_ap(k, ew):
        copy, band = divmod(k, 4)
        src = p0_d if copy == 0 else p1_d
        return AP(src, band * YP_BAND * W * UNIT, [[UNIT, YP_BAND * W],
                                                   [1, ew]])

    with tile.TileContext(nc) as tc:
        with tc.tile_pool(name="persist", bufs=1) as pp:
            idx0 = pp.tile([P, L // 16], I16)
            idx1 = pp.tile([P, L // 16], I16)
            wts = pp.tile([P, (L // P) * 4], F32)
            nc.sync.dma_start(idx0[:], idx0_d.ap())
            nc.sync.dma_start(idx1[:], idx1_d.ap())
            nc.sync.dma_start(wts[:], wts_d.ap())
            wtsv = wts[:].rearrange("p (m s) -> p m s", s=4)

            with tc.tile_pool(name="work", bufs=2) as wp:

                for _ in range(repeat):
                    for ci in range(nchunks):
                        if MERGED:
                            X0 = wp.tile([P, CHUNK], F32, tag="X0")
                            G0 = X0[:].rearrange("p (m e) -> p m e",
                                                 e=2 * UNIT)
                            streams = ((idx0, G0),)
                            ew = 2 * UNIT
                        else:
                            X0 = wp.tile([P, CHUNK // 2], F32, tag="X0")
                            X1 = wp.tile([P, CHUNK // 2], F32, tag="X1")
                            G0 = X0[:].rearrange("p (m e) -> p m e", e=UNIT)
                            G1 = X1[:].rearrange("p (m e) -> p m e", e=UNIT)
                            streams = ((idx0, G0), (idx1, G1))
                            ew = UNIT
                        for (k, s0, cnt) in segments[ci]:
                            gcol = (ci * CHUNK + s0) // 16
                            m0 = s0 // P
                            mw = cnt // P
                            jobs = []
                            nsplit = NQ // len(streams)
                            for qn, (idx_sb, dst) in enumerate(streams):
                                if nsplit >= 2 and mw >= nsplit:
                                    mh = mw // nsplit
                                    off = 0
                                    for j in range(nsplit):
                                        mj = (mw - off if j == nsplit - 1
                                              else mh)
                                        jobs.append(
                                            (qn + j * len(streams), idx_sb,
                                             dst, m0 + off,
                                             mj, gcol + (off * P) // 16))
                                        off += mj
                                else:
                                    jobs.append((qn, idx_sb, dst, m0, mw,
                                                 gcol))
                            for (q, idx_sb, dst, mm0, mmw, gc) in jobs:
                                nc.gpsimd.dma_gather(
                                    out_ap=dst[:, mm0:mm0 + mmw, :],
                                    in_ap=class_in_ap(k, ew),
                                    idxs_ap=idx_sb[:, gc:gc
                                                   + (mmw * P) // 16],
                                    num_idxs=mmw * P,
                                    num_idxs_reg=mmw * P,
                                    elem_size=ew,
                                    elem_step=UNIT,
                                    single_packet=SINGLE_PACKET,
                                    queue_num=q % NQ,
                                )

                        res = wp.tile([P, MCOL * C], F32, tag="res")
                        tmp = wp.tile([P, MCOL * C], F32, tag="tmp")
                        r3 = res[:].rearrange("p (m e) -> p m e", e=C)
                        t3 = tmp[:].rearrange("p (m e) -> p m e", e=C)

                        def wb(s):
                            return (wtsv[:, ci * MCOL:(ci + 1) * MCOL,
                                         s:s + 1]
                                    .to_broadcast([P, MCOL, C]))

                        if MERGED:
                            srcs = [(G0, 0, 0), (G0, 1, 1), (G0, 2, 2),
                                    (G0, 3, 3)]
                        else:
                            srcs = [(G0, 0, 0), (G0, 1, 1), (G1, 0, 2),
                                    (G1, 1, 3)]
                        first = True
                        for (G3, half, s) in srcs:
                            sl = G3[:, :, half * C:half * C + C]
                            if first:
                                nc.vector.tensor_tensor(r3, sl, wb(s),
                                                        op=OP.mult)
                                first = False
                            else:
                                nc.vector.tensor_tensor(t3, sl, wb(s),
                                                        op=OP.mult)
                                nc.vector.tensor_tensor(r3, r3, t3,
                                                        op=OP.add)
                        nc.sync.dma_start(outv[ci], res[:])

    nc.compile()
    return nc


_NC_CACHE = {}


def kernel(images, boxes):
    images = np.ascontiguousarray(np.asarray(images, dtype=np.float32))
    boxes = np.ascontiguousarray(np.asarray(boxes, dtype=np.float32))
    B = images.shape[0]

    in_maps, segments, unsort_rows, L = _make_schedule(images, boxes)

    key = (B, L, tuple(tuple(s) for cs in segments for s in cs))
    nc = _NC_CACHE.get(key)
    if nc is None:
        nc = _build_nc(segments, L, num_devices=B)
        _NC_CACHE.clear()
        _NC_CACHE[key] = nc

    from concourse import bass_utils
    res = bass_utils.run_bass_kernel_spmd(nc, in_maps, core_ids=list(range(B)))

    outs = []
    for c in range(B):
        scratch = res.results[c]["out"]
        outs.append(scratch[unsort_rows[c]].reshape(NB, CROP, CROP, C))
    return np.concatenate(outs, axis=0)
